# revision 1
# baseline (speedup 1.0000x reference)
"""Trainium2 Bass kernel for a 4-term video/query contrastive loss.

Strategy: data-parallel over batch B=64 across 8 cores (8 videos/core).
The dominant work is contrasting every query / top-k feature against every
upper-triangular 2d-map proposal feature of every video:

  - host compacts video_feats to the 2080 triu columns (padded to 2560)
    and casts to bf16; weights = [64 queries | 16 local topk feats],
    L2-normalized on host, also bf16
  - scores are computed TRANSPOSED, S^T[p, r] (proposals on partitions),
    so the per-proposal 1/||v_p|| becomes a per-partition scale that the
    scalar engine applies inside exp() for free
  - per-proposal squared norms: matmul of V*V against a ones column
  - exp sums and the two iou-masked sums come from one tiny N=3 matmul
    per chunk with rhs = [valid_mask | pos_mask | neg_mask], accumulated
    in PSUM across each video's 20 chunks
  - the kernel is emitted in two phases (all Sqrt, then all Exp) so the
    scalar engine loads each activation LUT exactly once

Host (numpy) does index prep (top-k gather, masks, normalization of the
80 weight rows) and final log/mean assembly over ~10^3 scalars.
"""

import numpy as np
import ml_dtypes

import concourse.bacc as bacc
import concourse.bass as bass
import concourse.tile as tile
from concourse import mybir
from concourse import bass_utils

f32 = mybir.dt.float32
bf16 = mybir.dt.bfloat16
AOT = mybir.AluOpType
AFT = mybir.ActivationFunctionType
BF = ml_dtypes.bfloat16

B, C, D = 64, 256, 64
SP = D * D                 # 4096 flattened 2d-map positions
NTRIU = D * (D + 1) // 2   # 2080 upper-tri positions
SPP = 2560                 # compacted + padded proposal count (20 x 128)
NCORES = 8
VB = B // NCORES           # videos per core: 8
NPT = 2                    # sentences (num_targets) per video
NTC = NPT * VB             # sentences per core: 16
M80 = B + NTC              # score rows: 64 queries + 16 local topk feats
T = B * NPT                # 128 sentences
SMW = B + NTC + T          # 208 small-feature columns
PCH = 128                  # proposals per chunk (partition dim of S^T)
NPC = SPP // PCH           # 20 chunks per video
TAU_I = 10.0               # 1/temperature (T_V == T_Q == 0.1)
NEG_IOU = 0.5


def _build_module():
    nc = bacc.Bacc("TRN2", target_bir_lowering=False, debug=False)

    d_v = nc.dram_tensor("v16", (VB * C, SPP), bf16, kind="ExternalInput")
    d_w = nc.dram_tensor("w16", (C, M80), bf16, kind="ExternalInput")
    d_sf = nc.dram_tensor("sf", (C, SMW), f32, kind="ExternalInput")
    d_msk = nc.dram_tensor("msk", (VB * PCH, NPC * 3), bf16, kind="ExternalInput")
    d_bd = nc.dram_tensor("bd", (B, T), f32, kind="ExternalInput")
    d_qrn = nc.dram_tensor("qrn", (B, 1), f32, kind="ExternalInput")
    d_trn = nc.dram_tensor("trn", (NTC, 1), f32, kind="ExternalInput")

    d_osmall = nc.dram_tensor("o_small", (B, NTC + T), f32, kind="ExternalOutput")
    d_oa3 = nc.dram_tensor("o_a3", (NTC, NTC), f32, kind="ExternalOutput")
    d_ocs1 = nc.dram_tensor("o_cs1", (1, NTC), f32, kind="ExternalOutput")
    d_os4 = nc.dram_tensor("o_s4", (B, 2), f32, kind="ExternalOutput")
    d_or = nc.dram_tensor("o_r", (M80, VB * 3), f32, kind="ExternalOutput")

    with tile.TileContext(nc) as tc:
        with (
            tc.tile_pool(name="consts", bufs=1) as cp,
            tc.tile_pool(name="smalls", bufs=1) as sm,
            tc.tile_pool(name="sq", bufs=4) as sqp,
            tc.tile_pool(name="etile", bufs=4) as ep,
            tc.tile_pool(name="outs", bufs=1) as op_,
            tc.tile_pool(name="pn", bufs=2, space="PSUM") as pn,
            tc.tile_pool(name="ps", bufs=4, space="PSUM") as ps,
            tc.tile_pool(name="pr", bufs=2, space="PSUM") as pr,
        ):
            # ---- resident inputs ----
            vts = []
            for v in range(VB):
                for k in range(2):
                    vt = cp.tile([128, SPP], bf16, tag=f"v{v}_{k}")
                    nc.sync.dma_start(vt, d_v[v * C + k * 128:v * C + (k + 1) * 128, :])
                    vts.append(vt)
            w0 = cp.tile([128, M80], bf16, tag="w0")
            w1 = cp.tile([128, M80], bf16, tag="w1")
            nc.sync.dma_start(w0, d_w[0:128, :])
            nc.sync.dma_start(w1, d_w[128:256, :])
            sf0 = cp.tile([128, SMW], f32, tag="sf0")
            sf1 = cp.tile([128, SMW], f32, tag="sf1")
            nc.sync.dma_start(sf0, d_sf[0:128, :])
            nc.sync.dma_start(sf1, d_sf[128:256, :])
            msk_t = cp.tile([PCH, VB * NPC * 3], bf16, tag="msk")
            for v in range(VB):
                nc.sync.dma_start(msk_t[:, v * NPC * 3:(v + 1) * NPC * 3],
                                  d_msk[v * PCH:(v + 1) * PCH, :])
            bd_t = cp.tile([B, T], f32, tag="bd")
            nc.sync.dma_start(bd_t, d_bd[:])
            qrn = cp.tile([B, 1], f32, tag="qrn")
            nc.sync.dma_start(qrn, d_qrn[:])
            trn = cp.tile([NTC, 1], f32, tag="trn")
            nc.sync.dma_start(trn, d_trn[:])
            ones_t = cp.tile([128, 1], f32, tag="ones")
            nc.vector.memset(ones_t, 1.0)
            ones16 = cp.tile([128, 1], bf16, tag="ones16")
            nc.vector.memset(ones16, 1.0)
            ones80 = cp.tile([1, M80], f32, tag="ones80")
            nc.vector.memset(ones80, 1.0)
            rm10 = cp.tile([128, VB * NPC], f32, tag="rm10")

            # ---- small phase, part 1 (ACT: Sqrt/Copy table) ----
            sqf = sm.tile([128, SMW], f32, tag="sqf")
            nps = pn.tile([1, SMW], f32, tag="np")
            nc.vector.tensor_mul(sqf, sf0, sf0)
            nc.tensor.matmul(nps, ones_t, sqf, start=True, stop=False)
            sqf2 = sm.tile([128, SMW], f32, tag="sqf2")
            nc.vector.tensor_mul(sqf2, sf1, sf1)
            nc.tensor.matmul(nps, ones_t, sqf2, start=False, stop=True)
            nsr = sm.tile([1, SMW], f32, tag="nsr")
            nc.scalar.sqrt(nsr, nps)
            nc.vector.tensor_scalar_max(nsr, nsr, 1e-12)
            rvec = sm.tile([1, SMW], f32, tag="rvec")
            nc.vector.reciprocal(rvec, nsr)
            # broadcast rvec cols [B:SMW] to 64 partitions via K=1 matmul
            rvb_ps = pr.tile([B, NTC + T], f32, tag="racc")
            nc.tensor.matmul(rvb_ps, ones80[:, 0:B], rvec[0:1, B:SMW],
                             start=True, stop=True)
            rvb = sm.tile([B, NTC + T], f32, tag="rvb")
            nc.scalar.copy(rvb, rvb_ps)

            # ---- big phase A: per-proposal 1/norms (ACT: Sqrt) ----
            for v in range(VB):
                for c in range(NPC):
                    sl = slice(c * PCH, (c + 1) * PCH)
                    col = v * NPC + c
                    sq0 = sqp.tile([128, PCH], bf16, tag="sq")
                    nc.vector.tensor_mul(sq0, vts[2 * v][:, sl], vts[2 * v][:, sl])
                    np_ = pn.tile([128, 1], f32, tag="np")
                    nc.tensor.matmul(np_, sq0, ones16, start=True, stop=False)
                    sq1 = sqp.tile([128, PCH], bf16, tag="sq")
                    nc.vector.tensor_mul(sq1, vts[2 * v + 1][:, sl],
                                         vts[2 * v + 1][:, sl])
                    nc.tensor.matmul(np_, sq1, ones16, start=False, stop=True)
                    nr = sqp.tile([128, 1], f32, tag="nr")
                    nc.scalar.sqrt(nr, np_)
                    nc.vector.tensor_scalar_max(nr, nr, 1e-12)
                    nc.vector.reciprocal(rm10[:, col:col + 1], nr)
            nc.vector.tensor_scalar_mul(rm10, rm10, TAU_I)

            # ---- small phase, part 2 (ACT: Exp; before phase B) ----
            aps_ = ps.tile([B, NTC + T], f32, tag="st")
            nc.tensor.matmul(aps_, sf0[:, 0:B], sf0[:, B:SMW], start=True, stop=False)
            nc.tensor.matmul(aps_, sf1[:, 0:B], sf1[:, B:SMW], start=False, stop=True)
            nc.vector.tensor_scalar(out=aps_, in0=aps_, scalar1=qrn,
                                    scalar2=None, op0=AOT.mult)
            smalls = sm.tile([B, NTC + T], f32, tag="smalls")
            nc.vector.tensor_mul(smalls, aps_, rvb)
            nc.gpsimd.dma_start(d_osmall[:], smalls)

            a3ps = ps.tile([NTC, NTC], f32, tag="st")
            nc.tensor.matmul(a3ps, sf0[:, B:B + NTC], sf0[:, B:B + NTC],
                             start=True, stop=False)
            nc.tensor.matmul(a3ps, sf1[:, B:B + NTC], sf1[:, B:B + NTC],
                             start=False, stop=True)
            nc.vector.tensor_scalar(out=a3ps, in0=a3ps, scalar1=trn,
                                    scalar2=None, op0=AOT.mult)
            a3s = sm.tile([NTC, NTC], f32, tag="a3s")
            nc.vector.tensor_mul(a3s, a3ps, rvb[0:NTC, 0:NTC])
            nc.gpsimd.dma_start(d_oa3[:], a3s)

            e1 = sm.tile([B, NTC], f32, tag="e1")
            nc.scalar.activation(e1, smalls[:, 0:NTC], AFT.Exp, scale=TAU_I)
            cs1ps = pn.tile([1, NTC], f32, tag="np")
            nc.tensor.matmul(cs1ps, ones_t[0:B, :], e1, start=True, stop=True)
            cs1s = sm.tile([1, NTC], f32, tag="cs1s")
            nc.vector.tensor_copy(cs1s, cs1ps)
            nc.gpsimd.dma_start(d_ocs1[:], cs1s)

            e4 = sm.tile([B, T], f32, tag="e4")
            s4o = sm.tile([B, 2], f32, tag="s4o")
            nc.scalar.activation(e4, smalls[:, NTC:], AFT.Exp, scale=TAU_I,
                                 accum_out=s4o[:, 0:1])
            nc.vector.tensor_mul(e4, e4, bd_t)
            nc.vector.tensor_reduce(s4o[:, 1:2], e4, mybir.AxisListType.X, AOT.add)
            nc.gpsimd.dma_start(d_os4[:], s4o)

            # ---- big phase B: scores, exp, masked sums (ACT: Exp) ----
            or_sb = op_.tile([M80, VB * 3], f32, tag="orsb")
            for v in range(VB):
                racc = pr.tile([M80, 3], f32, tag="racc")
                for c in range(NPC):
                    sl = slice(c * PCH, (c + 1) * PCH)
                    col = v * NPC + c
                    st_ = ps.tile([PCH, M80], f32, tag="st")
                    nc.tensor.matmul(st_, vts[2 * v][:, sl], w0,
                                     start=True, stop=False)
                    nc.tensor.matmul(st_, vts[2 * v + 1][:, sl], w1,
                                     start=False, stop=True)
                    et = ep.tile([PCH, M80], bf16, tag="et")
                    nc.scalar.activation(et, st_, AFT.Exp,
                                         scale=rm10[:, col:col + 1])
                    nc.tensor.matmul(racc, et,
                                     msk_t[:, col * 3:col * 3 + 3],
                                     start=(c == 0), stop=(c == NPC - 1))
                nc.vector.tensor_copy(or_sb[:, v * 3:(v + 1) * 3], racc)
            nc.gpsimd.dma_start(d_or[:], or_sb)

    nc.compile()
    return nc


_MODULE = None


def _get_module():
    global _MODULE
    if _MODULE is None:
        _MODULE = _build_module()
    return _MODULE


def kernel(video_feats, query_feats, sents_feats, iou2d, iou2ds, num_targets):
    video_feats = np.ascontiguousarray(np.asarray(video_feats, np.float32))
    query_feats = np.asarray(query_feats, np.float32)
    sents_feats = np.asarray(sents_feats, np.float32)
    iou2d = np.asarray(iou2d, np.float32)
    iou2ds = np.asarray(iou2ds, np.float32)
    nt = np.asarray(num_targets)
    assert video_feats.shape == (B, C, D, D) and sents_feats.shape == (T, C)
    assert (nt == NPT).all(), "kernel assumes uniform num_targets == 2"

    rows, cols = np.triu_indices(D)
    tri_lin = rows * D + cols                          # (2080,) row-major

    vf_flat = video_feats.reshape(B, C, SP)
    iou_flat = iou2d.reshape(B, SP)
    iouf = iou2ds.reshape(T, SP)[:, tri_lin]           # (T, 2080)
    pstar = tri_lin[np.argmax(iouf, axis=1)]           # top-1 pos per sentence
    scatter = np.repeat(np.arange(B), NPT)
    tvr = vf_flat[scatter, :, pstar]                   # (T, C) raw topk feats

    # compact triu columns, pad to SPP, cast bf16
    v16 = np.zeros((B, C, SPP), BF)
    v16[:, :, :NTRIU] = vf_flat[:, :, tri_lin].astype(BF)

    qT = np.ascontiguousarray(query_feats.T)           # (C, B)
    sT = np.ascontiguousarray(sents_feats.T)           # (C, T)
    bd = np.zeros((B, T), np.float32)
    bd[scatter, np.arange(T)] = 1.0
    qrn = 1.0 / np.maximum(np.linalg.norm(query_feats, axis=1), 1e-12)
    trn_all = 1.0 / np.maximum(np.linalg.norm(tvr, axis=1), 1e-12)  # (T,)
    qn = query_feats * qrn[:, None]                    # (B, C) normalized
    tvn = tvr * trn_all[:, None]                       # (T, C) normalized

    iou_tri = iou_flat[:, tri_lin]                     # (B, 2080)

    in_maps = []
    for k in range(NCORES):
        g0 = k * VB
        tv_loc = tvr[NPT * g0: NPT * g0 + NTC]         # (16, C) raw
        smallf = np.concatenate([qT, tv_loc.T, sT], axis=1)       # (C, 208)
        w16 = np.concatenate([qn, tvn[NPT * g0: NPT * g0 + NTC]],
                             axis=0).T.astype(BF)                 # (C, 80)
        msk = np.zeros((VB, SPP, 3), np.float32)
        for v in range(VB):
            g = g0 + v
            msk[v, :NTRIU, 0] = 1.0
            msk[v, :NTRIU, 1] = iou_tri[g] > NEG_IOU
            msk[v, :NTRIU, 2] = iou_tri[g] < NEG_IOU
        mskr = msk.reshape(VB, NPC, PCH, 3).transpose(0, 2, 1, 3).reshape(
            VB * PCH, NPC * 3).astype(BF)
        in_maps.append({
            "v16": v16[g0:g0 + VB].reshape(VB * C, SPP),
            "w16": np.ascontiguousarray(w16),
            "sf": np.ascontiguousarray(smallf),
            "msk": np.ascontiguousarray(mskr),
            "bd": bd,
            "qrn": qrn[:, None].astype(np.float32),
            "trn": trn_all[NPT * g0: NPT * g0 + NTC, None].astype(np.float32),
        })

    nc = _get_module()
    res = bass_utils.run_bass_kernel_spmd(nc, in_maps, core_ids=list(range(NCORES)))
    kernel._last = res
    outs = res.results

    # ---- host finalization (tiny, float64) ----
    E = np.float64
    smalls = [o["o_small"].astype(E) for o in outs]

    # L2 denominators: o_r col 3v+0 = sum exp over valid triu positions
    negq = np.zeros(B)
    for k in range(NCORES):
        r = outs[k]["o_r"][:B].astype(E)               # (64, 24)
        negq += r[:, 0::3].sum(axis=1)
    for b in range(B):
        k, v = b // VB, b % VB
        negq[b] -= float(outs[k]["o_r"][b, 3 * v + 1])

    t1 = np.empty(T)
    t2 = np.empty(T)
    t4 = np.empty(T)
    sm0 = smalls[0]
    s4 = outs[0]["o_s4"].astype(E)
    for t in range(T):
        b = scatter[t]
        k, v = b // VB, b % VB
        lc = NPT * v + (t - NPT * b)
        pos = smalls[k][b, lc]
        asum = float(outs[k]["o_cs1"][0, lc])
        t1[t] = -(pos * TAU_I - np.log(asum))
        t2[t] = -(pos * TAU_I - np.log(np.exp(pos * TAU_I) + negq[b]))
        pos4 = sm0[b, NTC + t]
        ns4 = s4[b, 0] - s4[b, 1]
        t4[t] = -(pos4 * TAU_I - np.log(np.exp(pos4 * TAU_I) + ns4))

    t3 = []
    for g in range(B):
        k, v = g // VB, g % VB
        a3 = outs[k]["o_a3"].astype(E)
        r = outs[k]["o_r"].astype(E)
        for i in range(NPT):
            ns = r[B + NPT * v + i, 3 * v + 2]
            for j in range(NPT):
                pd = a3[NPT * v + i, NPT * v + j]
                t3.append(-(pd * TAU_I - np.log(np.exp(pd * TAU_I) + ns)))

    return np.stack([t1.mean(), t2.mean(), np.mean(t3), t4.mean()]).astype(np.float32)



# revision 6
# speedup vs baseline: 4.2214x; 4.2214x over previous
"""Trainium2 Bass kernel for a 4-term video/query contrastive loss.

Strategy: data-parallel over batch B=64 across 8 cores (8 videos/core).
The only work that is material on hardware is contrasting every query /
top-k feature against every upper-triangular 2d-map proposal feature of
every video (64 x 2080 proposals x 256 ch), plus the exp() and the three
iou-masked sums over proposals. Everything O(B*T*C) or smaller (norms,
top-k gather, the query<->sentence terms, final log/mean assembly) is
done on the host in float64.

Device design (per core, 8 videos):
  - v8: per-video triu-compacted, L2-normalized, x16-scaled fp8e4m3
    video features [16 x 128, 2176] (17 chunks of 128 proposals, zero
    padded). Loaded in 4 big DMA slabs so compute starts early.
  - scores are computed TRANSPOSED per 128-proposal chunk:
    st[p, r] = v_chunk.T @ w  (w = 64 queries + 2 own topk feats, fp8,
    normalized x16), accumulated over the two C-halves in PSUM.
    17 chunks of a video pack into 3 PSUM banks (7+7+3 x 66 cols).
  - one Exp pass per 2-bank span on the scalar engine (big activations
    amortize the ~352-cycle fixed cost), scale = 10/256 folds the
    temperature and the two x16 quantization scales.
  - masked sums: matmul with the per-chunk [128, 3] mask (valid/pos/neg)
    as the *stationary* operand (3-column weight load is ~free), et as
    the moving operand, accumulated per video into a [3, 66] PSUM slice.
  - output: just the 8 x [3, 66] mask-sum table (7.7 KB). The host
    reassembles all four losses from it.
"""

import numpy as np
import ml_dtypes

import concourse.bacc as bacc
import concourse.bass as bass
import concourse.tile as tile
from concourse import mybir
from concourse import bass_utils

f32 = mybir.dt.float32
bf16 = mybir.dt.bfloat16
f8 = mybir.dt.float8e4
AFT = mybir.ActivationFunctionType
F8NP = ml_dtypes.float8_e4m3
BF = ml_dtypes.bfloat16

B, C, D = 64, 256, 64
NTRIU = D * (D + 1) // 2   # 2080 upper-tri positions
NCORES = 8
VB = B // NCORES           # videos per core: 8
NPT = 2                    # sentences per video
T = B * NPT                # 128 sentences
NW = B + NPT               # score cols per video: 64 queries + 2 own topk
PCH = 128                  # proposals per chunk
NCH = 17                   # chunks per video (2080 -> 2176 padded)
SPP = NCH * PCH            # 2176
TAU_I = 10.0
SCALE = TAU_I / 256.0      # fold temperature and the two x16 fp8 scales
NEG_IOU = 0.5
# chunk c -> (psum bank, slot): banks hold 7/7/3 chunks of 66 columns
_BANK = [(c // 7, c % 7) if c < 14 else (2, c - 14) for c in range(NCH)]


def _build_module():
    nc = bacc.Bacc("TRN2", target_bir_lowering=False, debug=False)

    d_v = nc.dram_tensor("v8", (16, 128, SPP), f8, kind="ExternalInput")
    d_w = nc.dram_tensor("w8", (2, 128, VB * NW), f8, kind="ExternalInput")
    d_m = nc.dram_tensor("msk", (128, VB * NCH * 3), bf16, kind="ExternalInput")
    d_o = nc.dram_tensor("oracc", (3, VB * NW), f32, kind="ExternalOutput")

    with tile.TileContext(nc) as tc:
        with (
            tc.tile_pool(name="consts", bufs=1) as cp,
            tc.tile_pool(name="et", bufs=2) as ep,
            tc.tile_pool(name="gps", bufs=2, space="PSUM") as gp,
            tc.tile_pool(name="rps", bufs=1, space="PSUM") as rp,
            tc.tile_pool(name="outs", bufs=1) as op_,
        ):
            # tiny dummy exp first: preloads the ACT Exp table (~2.7us)
            # under the input DMAs instead of on the critical path
            zz = cp.tile([1, 2], f32, tag="zz")
            nc.vector.memset(zz, 0.0)
            zz2 = cp.tile([1, 2], f32, tag="zz2")
            nc.scalar.activation(zz2, zz, AFT.Exp)

            # small inputs first, then video slabs in compute order
            msk_t = cp.tile([128, VB * NCH * 3], bf16, tag="msk")
            nc.sync.dma_start(msk_t, d_m[:])
            w0 = cp.tile([128, VB * NW], f8, tag="w0")
            w1 = cp.tile([128, VB * NW], f8, tag="w1")
            nc.sync.dma_start(w0, d_w[0])
            nc.sync.dma_start(w1, d_w[1])
            slabs = []
            for j in range(4):
                sj = cp.tile([128, 4, SPP], f8, tag=f"slab{j}")
                nc.sync.dma_start(sj, d_v[4 * j:4 * j + 4].transpose([1, 0, 2]))
                slabs.append(sj)

            raccA = rp.tile([3, 4 * NW], f32, tag="ra")
            raccB = rp.tile([3, 4 * NW], f32, tag="rb")
            orc = op_.tile([3, VB * NW], f32, tag="orc")

            for v in range(VB):
                slab = slabs[v // 2]
                b0 = 2 * (v % 2)
                wsl = slice(NW * v, NW * v + NW)
                g = gp.tile([128, 3, 512], f32, tag="g")
                for c in range(NCH):
                    bank, slot = _BANK[c]
                    osl = slice(66 * slot, 66 * slot + 66)
                    cs = slice(PCH * c, PCH * c + PCH)
                    nc.tensor.matmul(g[:, bank, osl], slab[:, b0, cs],
                                     w0[:, wsl], start=True, stop=False)
                    nc.tensor.matmul(g[:, bank, osl], slab[:, b0 + 1, cs],
                                     w1[:, wsl], start=False, stop=True)
                et = ep.tile([128, 3, 512], bf16, tag="et")
                nc.scalar.activation(et[:, 0:2, 0:462], g[:, 0:2, 0:462],
                                     AFT.Exp, scale=SCALE)
                nc.scalar.activation(et[:, 2:3, 0:198], g[:, 2:3, 0:198],
                                     AFT.Exp, scale=SCALE)
                racc = raccA if v < 4 else raccB
                co = NW * (v % 4)
                for c in range(NCH):
                    bank, slot = _BANK[c]
                    osl = slice(66 * slot, 66 * slot + 66)
                    msl = slice(3 * (NCH * v + c), 3 * (NCH * v + c) + 3)
                    nc.tensor.matmul(racc[:, co:co + NW], msk_t[:, msl],
                                     et[:, bank, osl],
                                     start=(c == 0), stop=(c == NCH - 1))
                if v == 3:
                    nc.vector.tensor_copy(orc[:, 0:4 * NW], raccA)
                if v == 7:
                    nc.vector.tensor_copy(orc[:, 4 * NW:8 * NW], raccB)
            nc.gpsimd.dma_start(d_o[:], orc)

    nc.compile()
    return nc


_MODULE = None


def _get_module():
    global _MODULE
    if _MODULE is None:
        _MODULE = _build_module()
    return _MODULE


def kernel(video_feats, query_feats, sents_feats, iou2d, iou2ds, num_targets):
    video_feats = np.asarray(video_feats, np.float32)
    query_feats = np.asarray(query_feats, np.float32)
    sents_feats = np.asarray(sents_feats, np.float32)
    iou2d = np.asarray(iou2d, np.float32)
    iou2ds = np.asarray(iou2ds, np.float32)
    nt = np.asarray(num_targets)
    assert video_feats.shape == (B, C, D, D) and sents_feats.shape == (T, C)
    assert (nt == NPT).all(), "kernel assumes uniform num_targets == 2"

    rows, cols = np.triu_indices(D)
    tri = rows * D + cols                               # (2080,) row-major

    vtri = video_feats.reshape(B, C, D * D)[:, :, tri]  # (B, C, 2080)
    nrm = np.sqrt(np.einsum('bcp,bcp->bp', vtri, vtri))
    nrm = np.maximum(nrm, 1e-12)
    vn8 = np.zeros((B, C, SPP), F8NP)
    vn8[:, :, :NTRIU] = (vtri * (16.0 / nrm)[:, None, :]).astype(F8NP)

    iouf = iou2ds.reshape(T, D * D)[:, tri]             # (T, 2080)
    scatter = np.repeat(np.arange(B), NPT)
    amax = np.argmax(iouf, axis=1)                      # top-1 triu idx
    tvr = vtri[scatter, :, amax]                        # (T, C) raw topk
    tvn = tvr / np.maximum(np.linalg.norm(tvr, axis=1, keepdims=True), 1e-12)
    qn = query_feats / np.maximum(
        np.linalg.norm(query_feats, axis=1, keepdims=True), 1e-12)
    sn = sents_feats / np.maximum(
        np.linalg.norm(sents_feats, axis=1, keepdims=True), 1e-12)

    iou_tri = iou2d.reshape(B, D * D)[:, tri]           # (B, 2080)
    m3 = np.zeros((B, SPP, 3), np.float32)
    m3[:, :NTRIU, 0] = 1.0
    m3[:, :NTRIU, 1] = iou_tri > NEG_IOU
    m3[:, :NTRIU, 2] = iou_tri < NEG_IOU

    qnT16 = np.ascontiguousarray((16.0 * qn).T.astype(F8NP))   # (C, 64)
    tvn16 = (16.0 * tvn).astype(F8NP)                          # (T, C)

    in_maps = []
    for k in range(NCORES):
        g0 = k * VB
        w8 = np.empty((C, VB * NW), F8NP)
        for v in range(VB):
            w8[:, NW * v:NW * v + B] = qnT16
            w8[:, NW * v + B:NW * v + NW] = \
                tvn16[NPT * (g0 + v):NPT * (g0 + v) + NPT].T
        mskk = m3[g0:g0 + VB].reshape(VB, NCH, PCH, 3) \
            .transpose(2, 0, 1, 3).reshape(PCH, VB * NCH * 3).astype(BF)
        in_maps.append({
            "v8": np.ascontiguousarray(
                vn8[g0:g0 + VB].reshape(16, 128, SPP)),
            "w8": np.ascontiguousarray(w8.reshape(2, 128, VB * NW)),
            "msk": np.ascontiguousarray(mskk),
        })

    nc = _get_module()
    res = bass_utils.run_bass_kernel_spmd(nc, in_maps, core_ids=list(range(NCORES)))
    kernel._last = res
    outs = res.results

    # ---- host finalization (tiny, float64) ----
    E = np.float64
    valid = np.empty((B, B), E)     # [query b, video g] sum over all props
    posm = np.empty((B, B), E)      # [query b, video g] sum over iou>0.5
    negt = np.empty(T, E)           # [topk row] sum over own video iou<0.5
    for k in range(NCORES):
        o = outs[k]["oracc"].astype(E)          # (3, 528)
        for v in range(VB):
            g = k * VB + v
            cs = NW * v
            valid[:, g] = o[0, cs:cs + B]
            posm[:, g] = o[1, cs:cs + B]
            negt[NPT * g:NPT * g + NPT] = o[2, cs + B:cs + NW]

    qn = qn.astype(E)
    tvn = tvn.astype(E)
    sn = sn.astype(E)

    M1 = tvn @ qn.T                                    # (T, B)
    pos_t = M1[np.arange(T), scatter]
    t1 = -(TAU_I * pos_t - np.log(np.exp(TAU_I * M1).sum(1)))

    negq = valid.sum(1) - posm[np.arange(B), np.arange(B)]
    t2 = -(TAU_I * pos_t
           - np.log(np.exp(TAU_I * pos_t) + negq[scatter]))

    t3 = []
    for g in range(B):
        tv = tvn[NPT * g:NPT * g + NPT]
        a2 = tv @ tv.T
        for i in range(NPT):
            ns = negt[NPT * g + i]
            for j in range(NPT):
                pd = a2[i, j]
                t3.append(-(TAU_I * pd - np.log(np.exp(TAU_I * pd) + ns)))

    QS = qn @ sn.T                                     # (B, T)
    EQ = np.exp(TAU_I * QS)
    row = EQ.sum(1)
    own = EQ[:, 0::2][np.arange(B), np.arange(B)] \
        + EQ[:, 1::2][np.arange(B), np.arange(B)]
    pos4 = QS[scatter, np.arange(T)]
    t4 = -(TAU_I * pos4
           - np.log(np.exp(TAU_I * pos4) + (row - own)[scatter]))

    return np.stack([t1.mean(), t2.mean(), np.mean(t3),
                     t4.mean()]).astype(np.float32)


# revision 11
# speedup vs baseline: 4.5551x; 1.0791x over previous
"""Trainium2 Bass kernel for a 4-term video/query contrastive loss.

Strategy: data-parallel over batch B=64 across 8 cores (8 videos/core).
The only work that is material on hardware is contrasting every query /
top-k feature against every upper-triangular 2d-map proposal feature of
every video (64 x 2080 proposals x 256 ch), plus the exp() and the three
iou-masked sums over proposals. Everything O(B*T*C) or smaller (norms,
top-k gather, the query<->sentence terms, final log/mean assembly) is
done on the host in float64.

Device design (per core, 8 videos):
  - v8: per-video triu-compacted, L2-normalized, x16-scaled fp8e4m3
    video features [16 x 128, 2176] (17 chunks of 128 proposals, zero
    padded). Loaded in 4 big DMA slabs so compute starts early.
  - scores are computed TRANSPOSED per 128-proposal chunk:
    st[p, r] = v_chunk.T @ w  (w = 64 queries + 2 own topk feats, fp8,
    normalized x16), accumulated over the two C-halves in PSUM.
    17 chunks of a video pack into 3 PSUM banks (7+7+3 x 66 cols).
  - one Exp pass per 2-bank span on the scalar engine (big activations
    amortize the ~352-cycle fixed cost), scale = 10/256 folds the
    temperature and the two x16 quantization scales.
  - masked sums: matmul with the per-chunk [128, 3] mask (valid/pos/neg)
    as the *stationary* operand (3-column weight load is ~free), et as
    the moving operand, accumulated per video into a [3, 66] PSUM slice.
  - output: just the 8 x [3, 66] mask-sum table (7.7 KB). The host
    reassembles all four losses from it.
"""

import numpy as np
import ml_dtypes

import concourse.bacc as bacc
import concourse.bass as bass
import concourse.tile as tile
from concourse import mybir
from concourse import bass_utils

f32 = mybir.dt.float32
bf16 = mybir.dt.bfloat16
f8 = mybir.dt.float8e4
AFT = mybir.ActivationFunctionType
F8NP = ml_dtypes.float8_e4m3
BF = ml_dtypes.bfloat16

B, C, D = 64, 256, 64
NTRIU = D * (D + 1) // 2   # 2080 upper-tri positions
NCORES = 8
VB = B // NCORES           # videos per core: 8
NPT = 2                    # sentences per video
T = B * NPT                # 128 sentences
NW = B + NPT               # score cols per video: 64 queries + 2 own topk
PCH = 128                  # proposals per chunk
NCH = 17                   # chunks per video (2080 -> 2176 padded)
SPP = NCH * PCH            # 2176
TAU_I = 10.0
SCALE = TAU_I / 256.0      # fold temperature and the two x16 fp8 scales
NEG_IOU = 0.5
# chunk c -> (psum bank, slot): banks hold 7/7/3 chunks of 66 columns
_BANK = [(c // 7, c % 7) if c < 14 else (2, c - 14) for c in range(NCH)]
# DMA slab sizes in blocks (block = one C-half of one video, 2 per video)
SLABS = (2, 6, 4, 4)


def _build_module():
    nc = bacc.Bacc("TRN2", target_bir_lowering=False, debug=False)

    d_v = nc.dram_tensor("v8", (128, 16, SPP), f8, kind="ExternalInput")
    d_w = nc.dram_tensor("w8", (2, 128, VB * NW), f8, kind="ExternalInput")
    d_m = nc.dram_tensor("msk", (128, VB * NCH * 3), bf16, kind="ExternalInput")
    d_o = nc.dram_tensor("oracc", (3, VB * NW), f32, kind="ExternalOutput")

    with tile.TileContext(nc) as tc:
        with (
            tc.tile_pool(name="consts", bufs=1) as cp,
            tc.tile_pool(name="et", bufs=2) as ep,
            tc.tile_pool(name="gps", bufs=2, space="PSUM") as gp,
            tc.tile_pool(name="rps", bufs=1, space="PSUM") as rp,
            tc.tile_pool(name="outs", bufs=1) as op_,
        ):
            # tiny dummy exp first: preloads the ACT Exp table (~2.7us)
            # under the input DMAs instead of on the critical path
            zz = cp.tile([1, 2], f32, tag="zz")
            nc.vector.memset(zz, 0.0)
            zz2 = cp.tile([1, 2], f32, tag="zz2")
            nc.scalar.activation(zz2, zz, AFT.Exp)

            # small inputs first, then video slabs in compute order
            msk_t = cp.tile([128, VB * NCH * 3], bf16, tag="msk")
            nc.sync.dma_start(msk_t, d_m[:])
            w0 = cp.tile([128, VB * NW], f8, tag="w0")
            w1 = cp.tile([128, VB * NW], f8, tag="w1")
            nc.sync.dma_start(w0, d_w[0])
            nc.sync.dma_start(w1, d_w[1])
            # slab j holds blocks SLABS[j] (block = one C-half of one video);
            # first slab is small so compute starts early
            slab_of = {}
            slabs = []
            b0 = 0
            for j, nblk in enumerate(SLABS):
                sj = cp.tile([128, nblk, SPP], f8, tag=f"slab{j}")
                nc.sync.dma_start(sj, d_v[:, b0:b0 + nblk, :])
                for m in range(b0, b0 + nblk):
                    slab_of[m] = (sj, m - b0)
                slabs.append(sj)
                b0 += nblk

            raccA = rp.tile([3, 4 * NW], f32, tag="ra")
            raccB = rp.tile([3, 4 * NW], f32, tag="rb")
            orc = op_.tile([3, VB * NW], f32, tag="orc")

            # software-pipelined: scores(v) are emitted before ACT(v-1) +
            # masks(v-1), so the (in-order) PE queue always has score work
            # while the scalar engine runs the previous video's exps
            ets = [None] * VB
            gs = [None] * VB

            def emit_scores(v):
                s0, m0 = slab_of[2 * v]
                s1, m1 = slab_of[2 * v + 1]
                wsl = slice(NW * v, NW * v + NW)
                g = gp.tile([128, 3, 512], f32, tag="g")
                gs[v] = g
                for c in range(NCH):
                    bank, slot = _BANK[c]
                    osl = slice(66 * slot, 66 * slot + 66)
                    cs = slice(PCH * c, PCH * c + PCH)
                    nc.tensor.matmul(g[:, bank, osl], s0[:, m0, cs],
                                     w0[:, wsl], start=True, stop=False)
                    nc.tensor.matmul(g[:, bank, osl], s1[:, m1, cs],
                                     w1[:, wsl], start=False, stop=True)

            def emit_exp_masks(v):
                g = gs[v]
                et = ep.tile([128, 3, 512], bf16, tag="et")
                nc.scalar.activation(et[:, 0:2, 0:462], g[:, 0:2, 0:462],
                                     AFT.Exp, scale=SCALE)
                nc.scalar.activation(et[:, 2:3, 0:198], g[:, 2:3, 0:198],
                                     AFT.Exp, scale=SCALE)
                racc = raccA if v < 4 else raccB
                co = NW * (v % 4)
                for c in range(NCH):
                    bank, slot = _BANK[c]
                    osl = slice(66 * slot, 66 * slot + 66)
                    msl = slice(3 * (NCH * v + c), 3 * (NCH * v + c) + 3)
                    nc.tensor.matmul(racc[:, co:co + NW], msk_t[:, msl],
                                     et[:, bank, osl],
                                     start=(c == 0), stop=(c == NCH - 1))
                if v == 3:
                    nc.vector.tensor_copy(orc[:, 0:4 * NW], raccA)
                    nc.sync.dma_start(d_o[:, 0:4 * NW], orc[:, 0:4 * NW])
                if v == 7:
                    nc.vector.tensor_copy(orc[:, 4 * NW:8 * NW], raccB)
                    nc.sync.dma_start(d_o[:, 4 * NW:8 * NW],
                                      orc[:, 4 * NW:8 * NW])

            for v in range(VB):
                emit_scores(v)
                if v > 0:
                    emit_exp_masks(v - 1)
            emit_exp_masks(VB - 1)

    nc.compile()
    return nc


_MODULE = None


def _get_module():
    global _MODULE
    if _MODULE is None:
        _MODULE = _build_module()
    return _MODULE


def kernel(video_feats, query_feats, sents_feats, iou2d, iou2ds, num_targets):
    video_feats = np.asarray(video_feats, np.float32)
    query_feats = np.asarray(query_feats, np.float32)
    sents_feats = np.asarray(sents_feats, np.float32)
    iou2d = np.asarray(iou2d, np.float32)
    iou2ds = np.asarray(iou2ds, np.float32)
    nt = np.asarray(num_targets)
    assert video_feats.shape == (B, C, D, D) and sents_feats.shape == (T, C)
    assert (nt == NPT).all(), "kernel assumes uniform num_targets == 2"

    rows, cols = np.triu_indices(D)
    tri = rows * D + cols                               # (2080,) row-major

    vtri = video_feats.reshape(B, C, D * D)[:, :, tri]  # (B, C, 2080)
    nrm = np.sqrt(np.einsum('bcp,bcp->bp', vtri, vtri))
    nrm = np.maximum(nrm, 1e-12)
    vn8 = np.zeros((B, C, SPP), F8NP)
    vn8[:, :, :NTRIU] = (vtri * (16.0 / nrm)[:, None, :]).astype(F8NP)

    iouf = iou2ds.reshape(T, D * D)[:, tri]             # (T, 2080)
    scatter = np.repeat(np.arange(B), NPT)
    amax = np.argmax(iouf, axis=1)                      # top-1 triu idx
    tvr = vtri[scatter, :, amax]                        # (T, C) raw topk
    tvn = tvr / np.maximum(np.linalg.norm(tvr, axis=1, keepdims=True), 1e-12)
    qn = query_feats / np.maximum(
        np.linalg.norm(query_feats, axis=1, keepdims=True), 1e-12)
    sn = sents_feats / np.maximum(
        np.linalg.norm(sents_feats, axis=1, keepdims=True), 1e-12)

    iou_tri = iou2d.reshape(B, D * D)[:, tri]           # (B, 2080)
    m3 = np.zeros((B, SPP, 3), np.float32)
    m3[:, :NTRIU, 0] = 1.0
    m3[:, :NTRIU, 1] = iou_tri > NEG_IOU
    m3[:, :NTRIU, 2] = iou_tri < NEG_IOU

    qnT16 = np.ascontiguousarray((16.0 * qn).T.astype(F8NP))   # (C, 64)
    tvn16 = (16.0 * tvn).astype(F8NP)                          # (T, C)

    in_maps = []
    for k in range(NCORES):
        g0 = k * VB
        w8 = np.empty((C, VB * NW), F8NP)
        for v in range(VB):
            w8[:, NW * v:NW * v + B] = qnT16
            w8[:, NW * v + B:NW * v + NW] = \
                tvn16[NPT * (g0 + v):NPT * (g0 + v) + NPT].T
        mskk = m3[g0:g0 + VB].reshape(VB, NCH, PCH, 3) \
            .transpose(2, 0, 1, 3).reshape(PCH, VB * NCH * 3).astype(BF)
        in_maps.append({
            "v8": np.ascontiguousarray(
                vn8[g0:g0 + VB].reshape(16, 128, SPP).transpose(1, 0, 2)),
            "w8": np.ascontiguousarray(w8.reshape(2, 128, VB * NW)),
            "msk": np.ascontiguousarray(mskk),
        })

    nc = _get_module()
    res = bass_utils.run_bass_kernel_spmd(nc, in_maps, core_ids=list(range(NCORES)))
    kernel._last = res
    outs = res.results

    # ---- host finalization (tiny, float64) ----
    E = np.float64
    valid = np.empty((B, B), E)     # [query b, video g] sum over all props
    posm = np.empty((B, B), E)      # [query b, video g] sum over iou>0.5
    negt = np.empty(T, E)           # [topk row] sum over own video iou<0.5
    for k in range(NCORES):
        o = outs[k]["oracc"].astype(E)          # (3, 528)
        for v in range(VB):
            g = k * VB + v
            cs = NW * v
            valid[:, g] = o[0, cs:cs + B]
            posm[:, g] = o[1, cs:cs + B]
            negt[NPT * g:NPT * g + NPT] = o[2, cs + B:cs + NW]

    qn = qn.astype(E)
    tvn = tvn.astype(E)
    sn = sn.astype(E)

    M1 = tvn @ qn.T                                    # (T, B)
    pos_t = M1[np.arange(T), scatter]
    t1 = -(TAU_I * pos_t - np.log(np.exp(TAU_I * M1).sum(1)))

    negq = valid.sum(1) - posm[np.arange(B), np.arange(B)]
    t2 = -(TAU_I * pos_t
           - np.log(np.exp(TAU_I * pos_t) + negq[scatter]))

    t3 = []
    for g in range(B):
        tv = tvn[NPT * g:NPT * g + NPT]
        a2 = tv @ tv.T
        for i in range(NPT):
            ns = negt[NPT * g + i]
            for j in range(NPT):
                pd = a2[i, j]
                t3.append(-(TAU_I * pd - np.log(np.exp(TAU_I * pd) + ns)))

    QS = qn @ sn.T                                     # (B, T)
    EQ = np.exp(TAU_I * QS)
    row = EQ.sum(1)
    own = EQ[:, 0::2][np.arange(B), np.arange(B)] \
        + EQ[:, 1::2][np.arange(B), np.arange(B)]
    pos4 = QS[scatter, np.arange(T)]
    t4 = -(TAU_I * pos4
           - np.log(np.exp(TAU_I * pos4) + (row - own)[scatter]))

    return np.stack([t1.mean(), t2.mean(), np.mean(t3),
                     t4.mean()]).astype(np.float32)


# revision 15
# speedup vs baseline: 4.6097x; 1.0120x over previous
"""Trainium2 Bass kernel for a 4-term video/query contrastive loss.

Strategy: data-parallel over batch B=64 across 8 cores (8 videos/core).
The only work that is material on hardware is contrasting every query /
top-k feature against every upper-triangular 2d-map proposal feature of
every video (64 x 2080 proposals x 256 ch), plus the exp() and the three
iou-masked sums over proposals. Everything O(B*T*C) or smaller (norms,
top-k gather, the query<->sentence terms, final log/mean assembly) is
done on the host in float64.

Device design (per core, 8 videos):
  - v8: per-video triu-compacted, L2-normalized, x16-scaled fp8e4m3
    video features [16 x 128, 2176] (17 chunks of 128 proposals, zero
    padded). Loaded in 4 big DMA slabs so compute starts early.
  - scores are computed TRANSPOSED per 128-proposal chunk:
    st[p, r] = v_chunk.T @ w  (w = 64 queries + 2 own topk feats, fp8,
    normalized x16), accumulated over the two C-halves in PSUM.
    17 chunks of a video pack into 3 PSUM banks (7+7+3 x 66 cols).
  - one Exp pass per 2-bank span on the scalar engine (big activations
    amortize the ~352-cycle fixed cost), scale = 10/256 folds the
    temperature and the two x16 quantization scales.
  - masked sums: matmul with the per-chunk [128, 3] mask (valid/pos/neg)
    as the *stationary* operand (3-column weight load is ~free), et as
    the moving operand, accumulated per video into a [3, 66] PSUM slice.
  - output: just the 8 x [3, 66] mask-sum table (7.7 KB). The host
    reassembles all four losses from it.
"""

import numpy as np
import ml_dtypes

import concourse.bacc as bacc
import concourse.bass as bass
import concourse.tile as tile
from concourse import mybir
from concourse import bass_utils

f32 = mybir.dt.float32
bf16 = mybir.dt.bfloat16
f8 = mybir.dt.float8e4
AFT = mybir.ActivationFunctionType
F8NP = ml_dtypes.float8_e4m3
BF = ml_dtypes.bfloat16

B, C, D = 64, 256, 64
NTRIU = D * (D + 1) // 2   # 2080 upper-tri positions
NCORES = 8
VB = B // NCORES           # videos per core: 8
NPT = 2                    # sentences per video
T = B * NPT                # 128 sentences
NW = B + NPT               # score cols per video: 64 queries + 2 own topk
PCH = 128                  # proposals per chunk
NCH = 17                   # chunks per video (2080 -> 2176 padded)
SPP = NCH * PCH            # 2176
TAU_I = 10.0
SCALE = TAU_I / 256.0      # fold temperature and the two x16 fp8 scales
NEG_IOU = 0.5
# chunk c -> (psum bank, slot): banks hold 7/7/3 chunks of 66 columns
_BANK = [(c // 7, c % 7) if c < 14 else (2, c - 14) for c in range(NCH)]
# DMA slab sizes in blocks (block = one C-half of one video, 2 per video)
SLABS = (2, 2, 4, 4, 4)


def _build_module():
    nc = bacc.Bacc("TRN2", target_bir_lowering=False, debug=False)

    d_v = nc.dram_tensor("v8", (128, 16, SPP), f8, kind="ExternalInput")
    d_w = nc.dram_tensor("w8", (128, 2, VB * NW), f8, kind="ExternalInput")
    d_m = nc.dram_tensor("msk", (128, VB * NCH * 3), bf16, kind="ExternalInput")
    d_o = nc.dram_tensor("oracc", (3, VB * NW), f32, kind="ExternalOutput")

    with tile.TileContext(nc) as tc:
        with (
            tc.tile_pool(name="consts", bufs=1) as cp,
            tc.tile_pool(name="et", bufs=2) as ep,
            tc.tile_pool(name="gps", bufs=2, space="PSUM") as gp,
            tc.tile_pool(name="rps", bufs=1, space="PSUM") as rp,
            tc.tile_pool(name="outs", bufs=1) as op_,
        ):
            # tiny dummy exp first: preloads the ACT Exp table (~2.7us)
            # under the input DMAs instead of on the critical path
            zz = cp.tile([1, 2], f32, tag="zz")
            nc.vector.memset(zz, 0.0)
            zz2 = cp.tile([1, 2], f32, tag="zz2")
            nc.scalar.activation(zz2, zz, AFT.Exp)

            # DMA order drives time-to-first-matmul: weights, then the
            # first (small) video slab, then the masks (first needed a
            # couple of microseconds into compute), then the rest
            wt = cp.tile([128, 2, VB * NW], f8, tag="wt")
            nc.sync.dma_start(wt, d_w[:])
            w0 = wt[:, 0, :]
            w1 = wt[:, 1, :]
            slab_of = {}
            slabs = []
            b0 = 0
            for j, nblk in enumerate(SLABS):
                sj = cp.tile([128, nblk, SPP], f8, tag=f"slab{j}")
                nc.sync.dma_start(sj, d_v[:, b0:b0 + nblk, :])
                for m in range(b0, b0 + nblk):
                    slab_of[m] = (sj, m - b0)
                slabs.append(sj)
                b0 += nblk
                if j == 0:
                    msk_t = cp.tile([128, VB * NCH * 3], bf16, tag="msk")
                    nc.sync.dma_start(msk_t, d_m[:])

            raccA = rp.tile([3, 4 * NW], f32, tag="ra")
            raccB = rp.tile([3, 4 * NW], f32, tag="rb")
            orc = op_.tile([3, VB * NW], f32, tag="orc")

            # software-pipelined: scores(v) are emitted before ACT(v-1) +
            # masks(v-1), so the (in-order) PE queue always has score work
            # while the scalar engine runs the previous video's exps
            ets = [None] * VB
            gs = [None] * VB

            def emit_scores(v):
                s0, m0 = slab_of[2 * v]
                s1, m1 = slab_of[2 * v + 1]
                wsl = slice(NW * v, NW * v + NW)
                g = gp.tile([128, 3, 512], f32, tag="g")
                gs[v] = g
                for c in range(NCH):
                    bank, slot = _BANK[c]
                    osl = slice(66 * slot, 66 * slot + 66)
                    cs = slice(PCH * c, PCH * c + PCH)
                    nc.tensor.matmul(g[:, bank, osl], s0[:, m0, cs],
                                     w0[:, wsl], start=True, stop=False)
                    nc.tensor.matmul(g[:, bank, osl], s1[:, m1, cs],
                                     w1[:, wsl], start=False, stop=True)

            def emit_exp_masks(v):
                g = gs[v]
                et = ep.tile([128, 3, 512], bf16, tag="et")
                nc.scalar.activation(et[:, 0:2, 0:462], g[:, 0:2, 0:462],
                                     AFT.Exp, scale=SCALE)
                nc.scalar.activation(et[:, 2:3, 0:198], g[:, 2:3, 0:198],
                                     AFT.Exp, scale=SCALE)
                racc = raccA if v < 4 else raccB
                co = NW * (v % 4)
                for c in range(NCH):
                    bank, slot = _BANK[c]
                    osl = slice(66 * slot, 66 * slot + 66)
                    msl = slice(3 * (NCH * v + c), 3 * (NCH * v + c) + 3)
                    nc.tensor.matmul(racc[:, co:co + NW], msk_t[:, msl],
                                     et[:, bank, osl],
                                     start=(c == 0), stop=(c == NCH - 1))
                if v == 3:
                    nc.vector.tensor_copy(orc[:, 0:4 * NW], raccA)
                    nc.sync.dma_start(d_o[:, 0:4 * NW], orc[:, 0:4 * NW])
                if v == 7:
                    nc.vector.tensor_copy(orc[:, 4 * NW:8 * NW], raccB)
                    nc.sync.dma_start(d_o[:, 4 * NW:8 * NW],
                                      orc[:, 4 * NW:8 * NW])

            for v in range(VB):
                emit_scores(v)
                if v > 0:
                    emit_exp_masks(v - 1)
            emit_exp_masks(VB - 1)

    nc.compile()
    return nc


_MODULE = None


def _get_module():
    global _MODULE
    if _MODULE is None:
        _MODULE = _build_module()
    return _MODULE


def kernel(video_feats, query_feats, sents_feats, iou2d, iou2ds, num_targets):
    video_feats = np.asarray(video_feats, np.float32)
    query_feats = np.asarray(query_feats, np.float32)
    sents_feats = np.asarray(sents_feats, np.float32)
    iou2d = np.asarray(iou2d, np.float32)
    iou2ds = np.asarray(iou2ds, np.float32)
    nt = np.asarray(num_targets)
    assert video_feats.shape == (B, C, D, D) and sents_feats.shape == (T, C)
    assert (nt == NPT).all(), "kernel assumes uniform num_targets == 2"

    rows, cols = np.triu_indices(D)
    tri = rows * D + cols                               # (2080,) row-major

    vtri = video_feats.reshape(B, C, D * D)[:, :, tri]  # (B, C, 2080)
    nrm = np.sqrt(np.einsum('bcp,bcp->bp', vtri, vtri))
    nrm = np.maximum(nrm, 1e-12)
    vn8 = np.zeros((B, C, SPP), F8NP)
    vn8[:, :, :NTRIU] = (vtri * (16.0 / nrm)[:, None, :]).astype(F8NP)

    iouf = iou2ds.reshape(T, D * D)[:, tri]             # (T, 2080)
    scatter = np.repeat(np.arange(B), NPT)
    amax = np.argmax(iouf, axis=1)                      # top-1 triu idx
    tvr = vtri[scatter, :, amax]                        # (T, C) raw topk
    tvn = tvr / np.maximum(np.linalg.norm(tvr, axis=1, keepdims=True), 1e-12)
    qn = query_feats / np.maximum(
        np.linalg.norm(query_feats, axis=1, keepdims=True), 1e-12)
    sn = sents_feats / np.maximum(
        np.linalg.norm(sents_feats, axis=1, keepdims=True), 1e-12)

    iou_tri = iou2d.reshape(B, D * D)[:, tri]           # (B, 2080)
    m3 = np.zeros((B, SPP, 3), np.float32)
    m3[:, :NTRIU, 0] = 1.0
    m3[:, :NTRIU, 1] = iou_tri > NEG_IOU
    m3[:, :NTRIU, 2] = iou_tri < NEG_IOU

    qnT16 = np.ascontiguousarray((16.0 * qn).T.astype(F8NP))   # (C, 64)
    tvn16 = (16.0 * tvn).astype(F8NP)                          # (T, C)

    in_maps = []
    for k in range(NCORES):
        g0 = k * VB
        w8 = np.empty((C, VB * NW), F8NP)
        for v in range(VB):
            w8[:, NW * v:NW * v + B] = qnT16
            w8[:, NW * v + B:NW * v + NW] = \
                tvn16[NPT * (g0 + v):NPT * (g0 + v) + NPT].T
        mskk = m3[g0:g0 + VB].reshape(VB, NCH, PCH, 3) \
            .transpose(2, 0, 1, 3).reshape(PCH, VB * NCH * 3).astype(BF)
        in_maps.append({
            "v8": np.ascontiguousarray(
                vn8[g0:g0 + VB].reshape(16, 128, SPP).transpose(1, 0, 2)),
            "w8": np.ascontiguousarray(
                w8.reshape(2, 128, VB * NW).transpose(1, 0, 2)),
            "msk": np.ascontiguousarray(mskk),
        })

    nc = _get_module()
    res = bass_utils.run_bass_kernel_spmd(nc, in_maps, core_ids=list(range(NCORES)))
    kernel._last = res
    outs = res.results

    # ---- host finalization (tiny, float64) ----
    E = np.float64
    valid = np.empty((B, B), E)     # [query b, video g] sum over all props
    posm = np.empty((B, B), E)      # [query b, video g] sum over iou>0.5
    negt = np.empty(T, E)           # [topk row] sum over own video iou<0.5
    for k in range(NCORES):
        o = outs[k]["oracc"].astype(E)          # (3, 528)
        for v in range(VB):
            g = k * VB + v
            cs = NW * v
            valid[:, g] = o[0, cs:cs + B]
            posm[:, g] = o[1, cs:cs + B]
            negt[NPT * g:NPT * g + NPT] = o[2, cs + B:cs + NW]

    qn = qn.astype(E)
    tvn = tvn.astype(E)
    sn = sn.astype(E)

    M1 = tvn @ qn.T                                    # (T, B)
    pos_t = M1[np.arange(T), scatter]
    t1 = -(TAU_I * pos_t - np.log(np.exp(TAU_I * M1).sum(1)))

    negq = valid.sum(1) - posm[np.arange(B), np.arange(B)]
    t2 = -(TAU_I * pos_t
           - np.log(np.exp(TAU_I * pos_t) + negq[scatter]))

    t3 = []
    for g in range(B):
        tv = tvn[NPT * g:NPT * g + NPT]
        a2 = tv @ tv.T
        for i in range(NPT):
            ns = negt[NPT * g + i]
            for j in range(NPT):
                pd = a2[i, j]
                t3.append(-(TAU_I * pd - np.log(np.exp(TAU_I * pd) + ns)))

    QS = qn @ sn.T                                     # (B, T)
    EQ = np.exp(TAU_I * QS)
    row = EQ.sum(1)
    own = EQ[:, 0::2][np.arange(B), np.arange(B)] \
        + EQ[:, 1::2][np.arange(B), np.arange(B)]
    pos4 = QS[scatter, np.arange(T)]
    t4 = -(TAU_I * pos4
           - np.log(np.exp(TAU_I * pos4) + (row - own)[scatter]))

    return np.stack([t1.mean(), t2.mean(), np.mean(t3),
                     t4.mean()]).astype(np.float32)


# revision 16
# speedup vs baseline: 5.2158x; 1.1315x over previous
"""Trainium2 Bass kernel for a 4-term video/query contrastive loss.

Strategy: data-parallel over batch B=64 across 8 cores (8 videos/core).
The only work that is material on hardware is contrasting every query /
top-k feature against every upper-triangular 2d-map proposal feature of
every video (64 x 2080 proposals x 256 ch), plus the exp() and the three
iou-masked sums over proposals. Everything O(B*T*C) or smaller (norms,
top-k gather, the query<->sentence terms, final log/mean assembly) is
done on the host in float64.

Device design (per core, 8 videos):
  - v8: per-video triu-compacted, L2-normalized, x16-scaled fp8e4m3
    video features [16 x 128, 2176] (17 chunks of 128 proposals, zero
    padded). Loaded in 4 big DMA slabs so compute starts early.
  - scores are computed TRANSPOSED per 128-proposal chunk:
    st[p, r] = v_chunk.T @ w  (w = 64 queries + 2 own topk feats, fp8,
    normalized x16), accumulated over the two C-halves in PSUM.
    17 chunks of a video pack into 3 PSUM banks (7+7+3 x 66 cols).
  - one Exp pass per 2-bank span on the scalar engine (big activations
    amortize the ~352-cycle fixed cost), scale = 10/256 folds the
    temperature and the two x16 quantization scales.
  - masked sums: matmul with the per-chunk [128, 3] mask (valid/pos/neg)
    as the *stationary* operand (3-column weight load is ~free), et as
    the moving operand, accumulated per video into a [3, 66] PSUM slice.
  - output: just the 8 x [3, 66] mask-sum table (7.7 KB). The host
    reassembles all four losses from it.
"""

import numpy as np
import ml_dtypes

import concourse.bacc as bacc
import concourse.bass as bass
import concourse.tile as tile
from concourse import mybir
from concourse import bass_utils

f32 = mybir.dt.float32
bf16 = mybir.dt.bfloat16
f8 = mybir.dt.float8e4
AFT = mybir.ActivationFunctionType
F8NP = ml_dtypes.float8_e4m3
BF = ml_dtypes.bfloat16

B, C, D = 64, 256, 64
NTRIU = D * (D + 1) // 2   # 2080 upper-tri positions
NCORES = 8
VB = B // NCORES           # videos per core: 8
NPT = 2                    # sentences per video
T = B * NPT                # 128 sentences
NW = B + NPT               # score cols per video: 64 queries + 2 own topk
PCH = 128                  # proposals per chunk
NCH = 17                   # chunks per video (2080 -> 2176 padded)
SPP = NCH * PCH            # 2176
TAU_I = 10.0
SCALE = TAU_I / 256.0      # fold temperature and the two x16 fp8 scales
NEG_IOU = 0.5
# chunk c -> (psum bank, slot): banks hold 7/7/3 chunks of 66 columns
_BANK = [(c // 7, c % 7) if c < 14 else (2, c - 14) for c in range(NCH)]
# DMA slab sizes in blocks (block = one C-half of one video, 2 per video)
SLABS = (2, 2, 4, 4, 4)


def _build_module():
    nc = bacc.Bacc("TRN2", target_bir_lowering=False, debug=False)

    d_v = nc.dram_tensor("v8", (128, 16, SPP), f8, kind="ExternalInput")
    d_w = nc.dram_tensor("w8", (128, 2, VB * NW), f8, kind="ExternalInput")
    d_m = nc.dram_tensor("msk", (128, VB * NCH * 3), bf16, kind="ExternalInput")
    d_o = nc.dram_tensor("oracc", (3, VB * NW), f32, kind="ExternalOutput")

    with tile.TileContext(nc) as tc:
        with (
            tc.tile_pool(name="consts", bufs=1) as cp,
            tc.tile_pool(name="et", bufs=2) as ep,
            tc.tile_pool(name="gps", bufs=2, space="PSUM") as gp,
            tc.tile_pool(name="rps", bufs=1, space="PSUM") as rp,
            tc.tile_pool(name="outs", bufs=1) as op_,
        ):
            # tiny dummy exp first: preloads the ACT Exp table (~2.7us)
            # under the input DMAs instead of on the critical path
            zz = cp.tile([1, 2], f32, tag="zz")
            nc.vector.memset(zz, 0.0)
            zz2 = cp.tile([1, 2], f32, tag="zz2")
            nc.scalar.activation(zz2, zz, AFT.Exp)

            # DMA order drives time-to-first-matmul: weights, then the
            # first (small) video slab, then the masks (first needed a
            # couple of microseconds into compute), then the rest
            # weights + masks ride the scalar HWDGE queue so the sync queue
            # starts streaming video slabs immediately
            wt = cp.tile([128, 2, VB * NW], f8, tag="wt")
            nc.scalar.dma_start(wt, d_w[:])
            w0 = wt[:, 0, :]
            w1 = wt[:, 1, :]
            msk_t = cp.tile([128, VB * NCH * 3], bf16, tag="msk")
            nc.scalar.dma_start(msk_t, d_m[:])
            slab_of = {}
            slabs = []
            b0 = 0
            for j, nblk in enumerate(SLABS):
                sj = cp.tile([128, nblk, SPP], f8, tag=f"slab{j}")
                nc.sync.dma_start(sj, d_v[:, b0:b0 + nblk, :])
                for m in range(b0, b0 + nblk):
                    slab_of[m] = (sj, m - b0)
                slabs.append(sj)
                b0 += nblk

            raccA = rp.tile([3, 4 * NW], f32, tag="ra")
            raccB = rp.tile([3, 4 * NW], f32, tag="rb")
            orc = op_.tile([3, VB * NW], f32, tag="orc")

            # software-pipelined: scores(v) are emitted before ACT(v-1) +
            # masks(v-1), so the (in-order) PE queue always has score work
            # while the scalar engine runs the previous video's exps
            ets = [None] * VB
            gs = [None] * VB

            def emit_scores(v):
                s0, m0 = slab_of[2 * v]
                s1, m1 = slab_of[2 * v + 1]
                wsl = slice(NW * v, NW * v + NW)
                g = gp.tile([128, 3, 512], f32, tag="g")
                gs[v] = g
                for c in range(NCH):
                    bank, slot = _BANK[c]
                    osl = slice(66 * slot, 66 * slot + 66)
                    cs = slice(PCH * c, PCH * c + PCH)
                    nc.tensor.matmul(g[:, bank, osl], s0[:, m0, cs],
                                     w0[:, wsl], start=True, stop=False)
                    nc.tensor.matmul(g[:, bank, osl], s1[:, m1, cs],
                                     w1[:, wsl], start=False, stop=True)

            def emit_exp_masks(v):
                g = gs[v]
                et = ep.tile([128, 3, 512], bf16, tag="et")
                nc.scalar.activation(et[:, 0:2, 0:462], g[:, 0:2, 0:462],
                                     AFT.Exp, scale=SCALE)
                nc.scalar.activation(et[:, 2:3, 0:198], g[:, 2:3, 0:198],
                                     AFT.Exp, scale=SCALE)
                racc = raccA if v < 4 else raccB
                co = NW * (v % 4)
                for c in range(NCH):
                    bank, slot = _BANK[c]
                    osl = slice(66 * slot, 66 * slot + 66)
                    msl = slice(3 * (NCH * v + c), 3 * (NCH * v + c) + 3)
                    nc.tensor.matmul(racc[:, co:co + NW], msk_t[:, msl],
                                     et[:, bank, osl],
                                     start=(c == 0), stop=(c == NCH - 1))
                if v == 3:
                    nc.vector.tensor_copy(orc[:, 0:4 * NW], raccA)
                    nc.sync.dma_start(d_o[:, 0:4 * NW], orc[:, 0:4 * NW])
                if v == 7:
                    nc.vector.tensor_copy(orc[:, 4 * NW:8 * NW], raccB)
                    nc.sync.dma_start(d_o[:, 4 * NW:8 * NW],
                                      orc[:, 4 * NW:8 * NW])

            for v in range(VB):
                emit_scores(v)
                if v > 0:
                    emit_exp_masks(v - 1)
            emit_exp_masks(VB - 1)

    nc.compile()
    return nc


_MODULE = None


def _get_module():
    global _MODULE
    if _MODULE is None:
        _MODULE = _build_module()
    return _MODULE


def kernel(video_feats, query_feats, sents_feats, iou2d, iou2ds, num_targets):
    video_feats = np.asarray(video_feats, np.float32)
    query_feats = np.asarray(query_feats, np.float32)
    sents_feats = np.asarray(sents_feats, np.float32)
    iou2d = np.asarray(iou2d, np.float32)
    iou2ds = np.asarray(iou2ds, np.float32)
    nt = np.asarray(num_targets)
    assert video_feats.shape == (B, C, D, D) and sents_feats.shape == (T, C)
    assert (nt == NPT).all(), "kernel assumes uniform num_targets == 2"

    rows, cols = np.triu_indices(D)
    tri = rows * D + cols                               # (2080,) row-major

    vtri = video_feats.reshape(B, C, D * D)[:, :, tri]  # (B, C, 2080)
    nrm = np.sqrt(np.einsum('bcp,bcp->bp', vtri, vtri))
    nrm = np.maximum(nrm, 1e-12)
    vn8 = np.zeros((B, C, SPP), F8NP)
    vn8[:, :, :NTRIU] = (vtri * (16.0 / nrm)[:, None, :]).astype(F8NP)

    iouf = iou2ds.reshape(T, D * D)[:, tri]             # (T, 2080)
    scatter = np.repeat(np.arange(B), NPT)
    amax = np.argmax(iouf, axis=1)                      # top-1 triu idx
    tvr = vtri[scatter, :, amax]                        # (T, C) raw topk
    tvn = tvr / np.maximum(np.linalg.norm(tvr, axis=1, keepdims=True), 1e-12)
    qn = query_feats / np.maximum(
        np.linalg.norm(query_feats, axis=1, keepdims=True), 1e-12)
    sn = sents_feats / np.maximum(
        np.linalg.norm(sents_feats, axis=1, keepdims=True), 1e-12)

    iou_tri = iou2d.reshape(B, D * D)[:, tri]           # (B, 2080)
    m3 = np.zeros((B, SPP, 3), np.float32)
    m3[:, :NTRIU, 0] = 1.0
    m3[:, :NTRIU, 1] = iou_tri > NEG_IOU
    m3[:, :NTRIU, 2] = iou_tri < NEG_IOU

    qnT16 = np.ascontiguousarray((16.0 * qn).T.astype(F8NP))   # (C, 64)
    tvn16 = (16.0 * tvn).astype(F8NP)                          # (T, C)

    in_maps = []
    for k in range(NCORES):
        g0 = k * VB
        w8 = np.empty((C, VB * NW), F8NP)
        for v in range(VB):
            w8[:, NW * v:NW * v + B] = qnT16
            w8[:, NW * v + B:NW * v + NW] = \
                tvn16[NPT * (g0 + v):NPT * (g0 + v) + NPT].T
        mskk = m3[g0:g0 + VB].reshape(VB, NCH, PCH, 3) \
            .transpose(2, 0, 1, 3).reshape(PCH, VB * NCH * 3).astype(BF)
        in_maps.append({
            "v8": np.ascontiguousarray(
                vn8[g0:g0 + VB].reshape(16, 128, SPP).transpose(1, 0, 2)),
            "w8": np.ascontiguousarray(
                w8.reshape(2, 128, VB * NW).transpose(1, 0, 2)),
            "msk": np.ascontiguousarray(mskk),
        })

    nc = _get_module()
    res = bass_utils.run_bass_kernel_spmd(nc, in_maps, core_ids=list(range(NCORES)))
    kernel._last = res
    outs = res.results

    # ---- host finalization (tiny, float64) ----
    E = np.float64
    valid = np.empty((B, B), E)     # [query b, video g] sum over all props
    posm = np.empty((B, B), E)      # [query b, video g] sum over iou>0.5
    negt = np.empty(T, E)           # [topk row] sum over own video iou<0.5
    for k in range(NCORES):
        o = outs[k]["oracc"].astype(E)          # (3, 528)
        for v in range(VB):
            g = k * VB + v
            cs = NW * v
            valid[:, g] = o[0, cs:cs + B]
            posm[:, g] = o[1, cs:cs + B]
            negt[NPT * g:NPT * g + NPT] = o[2, cs + B:cs + NW]

    qn = qn.astype(E)
    tvn = tvn.astype(E)
    sn = sn.astype(E)

    M1 = tvn @ qn.T                                    # (T, B)
    pos_t = M1[np.arange(T), scatter]
    t1 = -(TAU_I * pos_t - np.log(np.exp(TAU_I * M1).sum(1)))

    negq = valid.sum(1) - posm[np.arange(B), np.arange(B)]
    t2 = -(TAU_I * pos_t
           - np.log(np.exp(TAU_I * pos_t) + negq[scatter]))

    t3 = []
    for g in range(B):
        tv = tvn[NPT * g:NPT * g + NPT]
        a2 = tv @ tv.T
        for i in range(NPT):
            ns = negt[NPT * g + i]
            for j in range(NPT):
                pd = a2[i, j]
                t3.append(-(TAU_I * pd - np.log(np.exp(TAU_I * pd) + ns)))

    QS = qn @ sn.T                                     # (B, T)
    EQ = np.exp(TAU_I * QS)
    row = EQ.sum(1)
    own = EQ[:, 0::2][np.arange(B), np.arange(B)] \
        + EQ[:, 1::2][np.arange(B), np.arange(B)]
    pos4 = QS[scatter, np.arange(T)]
    t4 = -(TAU_I * pos4
           - np.log(np.exp(TAU_I * pos4) + (row - own)[scatter]))

    return np.stack([t1.mean(), t2.mean(), np.mean(t3),
                     t4.mean()]).astype(np.float32)
